# revision 11
# baseline (speedup 1.0000x reference)
"""Trainium2 Bass kernel for nn_DeepThermoMix (gnn_message_passing).

Math (reference):
    raw   = concat([comp_emb, mole_frac[:,None]], -1)            # [N, 129]
    pooled= segment_mean(raw, seg, M)                            # [M, 129]
    h     = softplus(pooled @ iW1 + ib1)                         # [M, 256]
    ctx   = softplus(h @ iW2 + ib2)                              # [M, 256]
    gin   = concat([comp_emb, mole_frac[:,None], ctx[seg]], -1)  # [N, 385]
    g     = softplus(gin @ gW1 + gb1)                            # [N, 256]
    latent= softplus(g @ gW2 + gb2)                              # [N, 256]
    return (latent, mole_frac)

Strategy (sharding_hint: data-parallel over mixtures):
  * Host: shard mixtures contiguously over 8 cores (12500 each), sort rows by
    mixture, greedily pack consecutive mixtures into "groups" of <=128
    mixtures and <=512 rows; pad each group to exactly 512 rows (zero rows)
    and 128 mixture slots.  All segment structure becomes two small 0/1
    masks per group, built on-device from the per-row local slot id:
       S[t][r, m] = (slot(r) == m)        (4 subtiles of 128 rows)
       E[m, r]    = (slot(r) == m)        (transposed layout)
  * Device per group (all matmuls fp32 data; fp32r fast path where N>=256):
       pooledT[k, m]  = sum_t xe[t].T @ S[t]       (segment sums, PE)
       q[m]           = sum_t S[t].T @ xf[t]       (mole-frac segment sums)
       h              = softplus(invc * (pooledT.T @ iW1e + q.T @ iW1f + cnt.T@ib1))
       ctx            = softplus(h @ iW2 + ib2)    (via PE transposes of h)
       ctxW           = ctx @ gW1[129:385]         (pre-multiplied expansion)
       g1T[k2, r]     = gW1e.T @ xeT + gW1f.T @ xf_row + ctxW.T @ E
       g1T            = softplus(g1T + gb1)
       latT[j, r]     = softplus(gW2.T-chunks @ g1T + gb2)
    latT written transposed; host transposes back and unpermutes rows.
"""

import numpy as np

# problem constants
M_TOT = 100000
N_TOT = 400000
EMB = 128
CTX = 256
LAT = 256
NCORES = 8
MC = M_TOT // NCORES          # mixtures per core
R_CAP = 512                   # rows per group (4 subtiles of 128)
M_CAP = 128                   # mixture slots per group
T_SUB = 4                     # row subtiles per group

F32 = np.float32


# ---------------------------------------------------------------- host packing
def _pack_core(counts_c):
    """Greedy pack consecutive mixtures into groups (<=M_CAP mixtures,
    <=R_CAP rows).  Returns (grp[mc], slot[mc], ngroups)."""
    mc = len(counts_c)
    grp = np.empty(mc, np.int64)
    slot = np.empty(mc, np.int64)
    g = 0
    rows = 0
    m = 0
    for i in range(mc):
        cnt = counts_c[i]
        if m + 1 > M_CAP or rows + cnt > R_CAP:
            g += 1
            rows = 0
            m = 0
        grp[i] = g
        slot[i] = m
        m += 1
        rows += cnt
    return grp, slot, g + 1


def _prepare_host(comp_emb, mole_frac, seg):
    """Shard + pack.  Returns (per_core list of dicts, per_core row indices,
    per_core dst positions, G)."""
    counts = np.bincount(seg, minlength=M_TOT).astype(np.int64)
    order = np.argsort(seg, kind="stable")
    csum = np.zeros(M_TOT + 1, np.int64)
    np.cumsum(counts, out=csum[1:])

    packs = []
    G = 0
    for c in range(NCORES):
        cnts = counts[c * MC:(c + 1) * MC]
        grp, slot, ng = _pack_core(cnts)
        packs.append((grp, slot, ng, cnts))
        G = max(G, ng)

    per_core = []
    scatter = []
    for c in range(NCORES):
        grp, slot, ng, cnts = packs[c]
        start, end = csum[c * MC], csum[(c + 1) * MC]
        rows_idx = order[start:end]                  # original rows, sorted
        mixl = seg[rows_idx] - c * MC                # local mixture id (sorted)
        g_row = grp[mixl]
        rows_per_group = np.bincount(g_row, minlength=G)
        assert rows_per_group.max() <= R_CAP
        gstart = np.zeros(G + 1, np.int64)
        np.cumsum(rows_per_group, out=gstart[1:])
        row_in_group = np.arange(len(rows_idx)) - gstart[g_row]
        dst = g_row * R_CAP + row_in_group           # padded position

        xe = np.zeros((G * R_CAP, EMB), F32)
        xe[dst] = comp_emb[rows_idx]
        xf = np.zeros(G * R_CAP, F32)
        xf[dst] = mole_frac[rows_idx]
        segl = np.zeros(G * R_CAP, F32)
        segl[dst] = slot[mixl].astype(F32)
        invc = np.zeros((G, M_CAP), F32)
        cntf = np.zeros((G, M_CAP), F32)
        nz = cnts > 0
        invc[grp[nz], slot[nz]] = (1.0 / cnts[nz]).astype(F32)
        cntf[grp, slot] = cnts.astype(F32)

        per_core.append(dict(
            xe=xe,
            xf=np.ascontiguousarray(xf.reshape(G, R_CAP)),
            segl=np.ascontiguousarray(segl.reshape(G, R_CAP)),
            invc=invc,
            cnt=cntf,
        ))
        scatter.append((rows_idx, dst))
    return per_core, scatter, G


# ---------------------------------------------------------------- bass program
SOFTPLUS_MODE = "expln"   # "native" (1-pass act2 table) or "expln" (2-pass)


def _patch_act_tables():
    """concourse's act-table map doesn't know the `act2` slot of the
    softplus_and_others set is softplus; teach it so bacc's table-load
    pass accepts ActivationFunctionType.Softplus."""
    import concourse.bacc as bacc_mod
    import concourse.mybir as mybir
    if getattr(bacc_mod, "_softplus_patched", False):
        return
    orig = bacc_mod.get_activation_tables

    def patched(arch):
        t = orig(arch)
        t["softplus_and_others"] = set(t["softplus_and_others"]) | {
            mybir.ActivationFunctionType.Softplus}
        return t

    bacc_mod.get_activation_tables = patched
    bacc_mod._softplus_patched = True


def _build_bass(G, use_pbcast=False):
    import concourse.bacc as bacc
    import concourse.mybir as mybir
    import concourse.tile as tile
    from concourse.masks import make_identity

    dt = mybir.dt
    f32 = dt.float32
    f32r = dt.float32r
    EQ = mybir.AluOpType.is_equal
    SP = mybir.ActivationFunctionType.Softplus
    if SOFTPLUS_MODE == "native":
        _patch_act_tables()

    def r(ap):  # fp32r view for fast fp32 matmuls
        return ap.bitcast(f32r)

    nc = bacc.Bacc(None, target_bir_lowering=False, debug=False)

    # ---- dram I/O
    xe_d = nc.dram_tensor("xe", [G * R_CAP, EMB], f32, kind="ExternalInput")
    xf_d = nc.dram_tensor("xf", [G, R_CAP], f32, kind="ExternalInput")
    segl_d = nc.dram_tensor("segl", [G, R_CAP], f32, kind="ExternalInput")
    invc_d = nc.dram_tensor("invc", [G, M_CAP], f32, kind="ExternalInput")
    cnt_d = nc.dram_tensor("cnt", [G, M_CAP], f32, kind="ExternalInput")
    iW1_d = nc.dram_tensor("iW1", [EMB + 1, CTX], f32, kind="ExternalInput")
    ib1_d = nc.dram_tensor("ib1", [1, CTX], f32, kind="ExternalInput")
    iW2_d = nc.dram_tensor("iW2", [CTX, CTX], f32, kind="ExternalInput")
    ib2_d = nc.dram_tensor("ib2", [1, CTX], f32, kind="ExternalInput")
    gW1_d = nc.dram_tensor("gW1", [EMB + 1 + CTX, LAT], f32, kind="ExternalInput")
    gb1_d = nc.dram_tensor("gb1", [LAT, 1], f32, kind="ExternalInput")
    gW2_d = nc.dram_tensor("gW2", [LAT, LAT], f32, kind="ExternalInput")
    gb2_d = nc.dram_tensor("gb2", [LAT, 1], f32, kind="ExternalInput")
    latT_d = nc.dram_tensor("latT", [LAT, G * R_CAP], f32, kind="ExternalOutput")

    with tile.TileContext(nc) as tc:
        with (
            tc.tile_pool(name="const", bufs=1) as cp,
            tc.tile_pool(name="io", bufs=3) as io,
            tc.tile_pool(name="work", bufs=3) as wp,
            tc.tile_pool(name="acc", bufs=2, space="PSUM") as pacc,
            tc.tile_pool(name="big", bufs=3, space="PSUM") as pbig,
            tc.tile_pool(name="tp", bufs=2, space="PSUM") as ptp,
        ):
            # ---- constants / weights
            def cload(name, src, shape):
                t = cp.tile(shape, f32, name=name)
                nc.sync.dma_start(out=t[:], in_=src)
                return t

            iW1e = cload("iW1e", iW1_d[0:EMB, :], [EMB, CTX])
            iW1f = cload("iW1f", iW1_d[EMB:EMB + 1, :], [1, CTX])
            ib1 = cload("ib1s", ib1_d[:, :], [1, CTX])
            iW2a = cload("iW2a", iW2_d[0:128, :], [128, CTX])
            iW2b = cload("iW2b", iW2_d[128:256, :], [128, CTX])
            ib2 = cload("ib2s", ib2_d[:, :], [1, CTX])
            gW1e = cload("gW1e", gW1_d[0:EMB, :], [EMB, LAT])
            gW1f = cload("gW1f", gW1_d[EMB:EMB + 1, :], [1, LAT])
            gW1c0 = cload("gW1c0", gW1_d[EMB + 1:EMB + 1 + 128, :], [128, LAT])
            gW1c1 = cload("gW1c1", gW1_d[EMB + 1 + 128:EMB + 1 + 256, :], [128, LAT])
            gb1c = [cload(f"gb1c{i}", gb1_d[i * 128:(i + 1) * 128, :], [128, 1])
                    for i in range(2)]
            gW2a = cload("gW2a", gW2_d[0:128, :], [128, LAT])
            gW2b = cload("gW2b", gW2_d[128:256, :], [128, LAT])
            gb2c = [cload(f"gb2c{i}", gb2_d[i * 128:(i + 1) * 128, :], [128, 1])
                    for i in range(2)]

            ones_row = cp.tile([1, 128], f32, name="ones_row")
            nc.gpsimd.memset(ones_row[:], 1.0)
            ident = cp.tile([128, 128], f32, name="ident")
            make_identity(nc, ident[:])

            iota_ci = cp.tile([128, 1], dt.int32, name="iota_ci")
            nc.gpsimd.iota(iota_ci[:], pattern=[[1, 1]], base=0,
                           channel_multiplier=1)
            iota_col = cp.tile([128, 1], f32, name="iota_col")
            nc.vector.tensor_copy(out=iota_col[:], in_=iota_ci[:])
            iota_ri = cp.tile([1, 128], dt.int32, name="iota_ri")
            nc.gpsimd.iota(iota_ri[:], pattern=[[1, 128]], base=0,
                           channel_multiplier=0)
            iota_row = cp.tile([1, 128], f32, name="iota_row")
            nc.vector.tensor_copy(out=iota_row[:], in_=iota_ri[:])

            if use_pbcast:
                iota_fb_ap = iota_row[:].partition_broadcast(128)
            else:
                ps_if = ptp.tile([128, 128], f32, name="ps_if", tag="tp")
                nc.tensor.matmul(out=ps_if[:], lhsT=r(ones_row[:]),
                                 rhs=r(iota_row[:]), start=True, stop=True)
                iota_fb = cp.tile([128, 128], f32, name="iota_fb")
                nc.vector.tensor_copy(out=iota_fb[:], in_=ps_if[:])
                iota_fb_ap = iota_fb[:]

            # ---- per-group stages, software-pipelined across groups so
            # every engine gets back-to-back work from adjacent groups.
            def stage1(g):
                """loads + masks + segment sums"""
                xe_sb = io.tile([128, R_CAP], f32r, name="xe_sb", tag="xe")
                for t in range(T_SUB):
                    nc.sync.dma_start(
                        out=xe_sb[:, t * 128:(t + 1) * 128],
                        in_=xe_d[g * R_CAP + t * 128: g * R_CAP + (t + 1) * 128, :])
                xf_col = io.tile([128, T_SUB], f32r, name="xf_col", tag="xfc")
                nc.sync.dma_start(
                    out=xf_col[:],
                    in_=xf_d.rearrange("g (t p) -> g p t", p=128)[g])
                xf_row = io.tile([1, R_CAP], f32r, name="xf_row", tag="xfr")
                nc.sync.dma_start(out=xf_row[:], in_=xf_d[g:g + 1, :])
                segl_col = io.tile([128, T_SUB], f32r, name="segl_col", tag="sgc")
                nc.sync.dma_start(
                    out=segl_col[:],
                    in_=segl_d.rearrange("g (t p) -> g p t", p=128)[g])
                segl_row = io.tile([1, R_CAP], f32r, name="segl_row", tag="sgr")
                nc.sync.dma_start(out=segl_row[:], in_=segl_d[g:g + 1, :])
                invc_col = io.tile([128, 1], f32, name="invc_col", tag="ivc")
                nc.sync.dma_start(out=invc_col[:], in_=invc_d[g, :, None])
                cnt_row = io.tile([1, 128], f32r, name="cnt_row", tag="cnt")
                nc.sync.dma_start(out=cnt_row[:], in_=cnt_d[g:g + 1, :])

                S = []
                for t in range(T_SUB):
                    s_t = wp.tile([128, 128], f32r, name=f"S{t}", tag=f"S{t}")
                    nc.vector.tensor_tensor(
                        out=s_t[:],
                        in0=segl_col[:, t:t + 1].to_broadcast([128, 128]),
                        in1=iota_fb_ap,
                        op=EQ)
                    S.append(s_t)
                E_sb = wp.tile([128, R_CAP], f32r, name="E_sb", tag="E")
                ps_segb = pacc.tile([128, R_CAP], f32, name="ps_segb", tag="acc")
                nc.tensor.matmul(out=ps_segb[:], lhsT=r(ones_row[:]),
                                 rhs=r(segl_row[:]), start=True, stop=True)
                nc.vector.tensor_tensor(
                    out=E_sb[:],
                    in0=ps_segb[:],
                    in1=iota_col[:].to_broadcast([128, R_CAP]),
                    op=EQ)

                ps_pool = pacc.tile([128, 128], f32, name="ps_pool", tag="acc")
                for t in range(T_SUB):
                    nc.tensor.matmul(out=ps_pool[:],
                                     lhsT=xe_sb[:, t * 128:(t + 1) * 128],
                                     rhs=S[t][:],
                                     start=(t == 0), stop=(t == T_SUB - 1))
                pooledT = wp.tile([128, 128], f32r, name="pooledT", tag="pooledT")
                nc.vector.tensor_copy(out=pooledT[:], in_=ps_pool[:])

                ps_q = pacc.tile([128, 1], f32, name="ps_q", tag="acc")
                for t in range(T_SUB):
                    # N=1 is illegal for fp32r matmul; run this one in fp32
                    nc.tensor.matmul(out=ps_q[:], lhsT=S[t][:].bitcast(f32),
                                     rhs=xf_col[:, t:t + 1].bitcast(f32),
                                     start=(t == 0), stop=(t == T_SUB - 1))
                q_sb = wp.tile([128, 1], f32r, name="q_sb", tag="q_sb")
                nc.vector.tensor_copy(out=q_sb[:], in_=ps_q[:])
                ps_qT = ptp.tile([1, 128], f32r, name="ps_qT", tag="tp")
                nc.tensor.transpose(out=ps_qT[:], in_=q_sb[:], identity=ident[:])
                qT_sb = wp.tile([1, 128], f32r, name="qT_sb", tag="qT_sb")
                nc.vector.tensor_copy(out=qT_sb[:], in_=ps_qT[:])
                return dict(xe_sb=xe_sb, xf_row=xf_row, E_sb=E_sb,
                            pooledT=pooledT, qT_sb=qT_sb,
                            invc_col=invc_col, cnt_row=cnt_row)

            def stage2a(g, d):
                """mixture mlp layer 1 (h)"""
                ps_h = pacc.tile([128, CTX], f32, name="ps_h", tag="acc")
                nc.tensor.matmul(out=ps_h[:], lhsT=r(d["pooledT"][:]),
                                 rhs=r(iW1e[:]), start=True, stop=False)
                nc.tensor.matmul(out=ps_h[:], lhsT=r(d["qT_sb"][:]),
                                 rhs=r(iW1f[:]), start=False, stop=False)
                nc.tensor.matmul(out=ps_h[:], lhsT=r(d["cnt_row"][:]),
                                 rhs=r(ib1[:]), start=False, stop=True)
                h_sb = wp.tile([128, CTX], f32r, name="h_sb", tag="h_sb")
                softplus(h_sb[:], ps_h[:], "h", scale=d["invc_col"][:])
                d["h_sb"] = h_sb
                return d

            def stage2b(g, d):
                """mixture mlp layer 2 (ctx, ctxW) + xeT transposes"""
                h_sb = d["h_sb"]
                hT = []
                for c in range(2):
                    ps_hT = ptp.tile([128, 128], f32r, name=f"ps_hT{c}", tag="tp")
                    nc.tensor.transpose(out=ps_hT[:],
                                        in_=h_sb[:, c * 128:(c + 1) * 128],
                                        identity=ident[:])
                    hT_c = wp.tile([128, 128], f32r, name=f"hT{c}", tag=f"hT{c}")
                    nc.vector.tensor_copy(out=hT_c[:], in_=ps_hT[:])
                    hT.append(hT_c)

                ps_ctx = pacc.tile([128, CTX], f32, name="ps_ctx", tag="acc")
                nc.tensor.matmul(out=ps_ctx[:], lhsT=r(hT[0][:]), rhs=r(iW2a[:]),
                                 start=True, stop=False)
                nc.tensor.matmul(out=ps_ctx[:], lhsT=r(hT[1][:]), rhs=r(iW2b[:]),
                                 start=False, stop=False)
                nc.tensor.matmul(out=ps_ctx[:], lhsT=r(ones_row[:]),
                                 rhs=r(ib2[:]), start=False, stop=True)
                ctx_sb = wp.tile([128, CTX], f32r, name="ctx_sb", tag="ctx_sb")
                softplus(ctx_sb[:], ps_ctx[:], "ctx")

                ctxT = []
                for c in range(2):
                    ps_cT = ptp.tile([128, 128], f32r, name=f"ps_cT{c}", tag="tp")
                    nc.tensor.transpose(out=ps_cT[:],
                                        in_=ctx_sb[:, c * 128:(c + 1) * 128],
                                        identity=ident[:])
                    cT = wp.tile([128, 128], f32r, name=f"ctxT{c}", tag=f"ctxT{c}")
                    nc.vector.tensor_copy(out=cT[:], in_=ps_cT[:])
                    ctxT.append(cT)
                ps_cW = pacc.tile([128, LAT], f32, name="ps_cW", tag="acc")
                nc.tensor.matmul(out=ps_cW[:], lhsT=r(ctxT[0][:]),
                                 rhs=r(gW1c0[:]), start=True, stop=False)
                nc.tensor.matmul(out=ps_cW[:], lhsT=r(ctxT[1][:]),
                                 rhs=r(gW1c1[:]), start=False, stop=True)
                ctxW = wp.tile([128, LAT], f32r, name="ctxW", tag="ctxW")
                nc.vector.tensor_copy(out=ctxW[:], in_=ps_cW[:])

                xeT = wp.tile([128, R_CAP], f32r, name="xeT", tag="xeT")
                for t in range(T_SUB):
                    ps_xT = ptp.tile([128, 128], f32r, name=f"ps_xT{t}", tag="tp")
                    nc.tensor.transpose(out=ps_xT[:],
                                        in_=d["xe_sb"][:, t * 128:(t + 1) * 128],
                                        identity=ident[:])
                    nc.vector.tensor_copy(out=xeT[:, t * 128:(t + 1) * 128],
                                          in_=ps_xT[:])
                return dict(xeT=xeT, ctxW=ctxW, xf_row=d["xf_row"],
                            E_sb=d["E_sb"])

            def stage3a(g, d):
                """gate layer; Exp per chunk (bias folded), one merged Ln"""
                g1_tmp = wp.tile([128, 2 * R_CAP], f32, name="g1_tmp",
                                 tag="sp_g1")
                for c2 in range(2):
                    ps_g1 = pbig.tile([128, R_CAP], f32, name=f"ps_g1_{c2}",
                                      tag="big")
                    nc.tensor.matmul(out=ps_g1[:],
                                     lhsT=r(gW1e[:, c2 * 128:(c2 + 1) * 128]),
                                     rhs=r(d["xeT"][:]), start=True, stop=False)
                    nc.tensor.matmul(out=ps_g1[:],
                                     lhsT=r(gW1f[:, c2 * 128:(c2 + 1) * 128]),
                                     rhs=r(d["xf_row"][:]), start=False,
                                     stop=False)
                    nc.tensor.matmul(out=ps_g1[:],
                                     lhsT=r(d["ctxW"][:, c2 * 128:(c2 + 1) * 128]),
                                     rhs=r(d["E_sb"][:]), start=False, stop=True)
                    nc.scalar.activation(
                        out=g1_tmp[:, c2 * R_CAP:(c2 + 1) * R_CAP],
                        in_=ps_g1[:],
                        func=mybir.ActivationFunctionType.Exp,
                        bias=gb1c[c2][:])
                g1T_pair = wp.tile([128, 2 * R_CAP], f32r, name="g1T_pair",
                                   tag="g1Tp")
                nc.scalar.activation(out=g1T_pair[:], in_=g1_tmp[:],
                                     func=mybir.ActivationFunctionType.Ln,
                                     bias=1.0)
                return dict(g1T=[g1T_pair[:, 0:R_CAP],
                                 g1T_pair[:, R_CAP:2 * R_CAP]])

            def stage3b(g, d):
                """latent + writeback; Exp per chunk, one merged Ln"""
                g1T = d["g1T"]
                lat_tmp = wp.tile([128, 2 * R_CAP], f32, name="lat_tmp",
                                  tag="sp_lat")
                for jc in range(2):
                    ps_lat = pbig.tile([128, R_CAP], f32, name=f"ps_lat{jc}",
                                       tag="big")
                    nc.tensor.matmul(out=ps_lat[:],
                                     lhsT=r(gW2a[:, jc * 128:(jc + 1) * 128]),
                                     rhs=r(g1T[0]), start=True, stop=False)
                    nc.tensor.matmul(out=ps_lat[:],
                                     lhsT=r(gW2b[:, jc * 128:(jc + 1) * 128]),
                                     rhs=r(g1T[1]), start=False, stop=True)
                    nc.scalar.activation(
                        out=lat_tmp[:, jc * R_CAP:(jc + 1) * R_CAP],
                        in_=ps_lat[:],
                        func=mybir.ActivationFunctionType.Exp,
                        bias=gb2c[jc][:])
                lat_pair = wp.tile([128, 2 * R_CAP], f32, name="lat_pair",
                                   tag="latp")
                nc.scalar.activation(out=lat_pair[:], in_=lat_tmp[:],
                                     func=mybir.ActivationFunctionType.Ln,
                                     bias=1.0)
                for jc in range(2):
                    nc.sync.dma_start(
                        out=latT_d[jc * 128:(jc + 1) * 128,
                                   g * R_CAP:(g + 1) * R_CAP],
                        in_=lat_pair[:, jc * R_CAP:(jc + 1) * R_CAP])

            st = [stage1, stage2a, stage2b, stage3a, stage3b]
            live = {}
            for i in range(G + 4):
                if i < G:
                    live[(i, 0)] = stage1(i)
                for s in (1, 2, 3):
                    g = i - s
                    if 0 <= g < G:
                        live[(g, s)] = st[s](g, live.pop((g, s - 1)))
                g = i - 4
                if 0 <= g < G:
                    stage3b(g, live.pop((g, 3)))

    nc.compile()
    return nc


# ------------------------------------------------------------------- kernel()
_CACHE = {}


def kernel(comp_emb, mole_frac, component_batch_batch,
           iW1, ib1, iW2, ib2, gW1, gb1, gW2, gb2):
    from concourse.bass_utils import run_bass_kernel_spmd

    comp_emb = np.ascontiguousarray(np.asarray(comp_emb, F32))
    mole_frac_np = np.ascontiguousarray(np.asarray(mole_frac, F32))
    seg = np.ascontiguousarray(np.asarray(component_batch_batch, np.int64))

    per_core, scatter, G = _prepare_host(comp_emb, mole_frac_np, seg)

    wts = dict(
        iW1=np.ascontiguousarray(np.asarray(iW1, F32)),
        ib1=np.ascontiguousarray(np.asarray(ib1, F32)).reshape(1, CTX),
        iW2=np.ascontiguousarray(np.asarray(iW2, F32)),
        ib2=np.ascontiguousarray(np.asarray(ib2, F32)).reshape(1, CTX),
        gW1=np.ascontiguousarray(np.asarray(gW1, F32)),
        gb1=np.ascontiguousarray(np.asarray(gb1, F32)).reshape(LAT, 1),
        gW2=np.ascontiguousarray(np.asarray(gW2, F32)),
        gb2=np.ascontiguousarray(np.asarray(gb2, F32)).reshape(LAT, 1),
    )

    if G not in _CACHE:
        _CACHE[G] = _build_bass(G)
    nc = _CACHE[G]

    in_maps = [{**pc, **wts} for pc in per_core]
    res = run_bass_kernel_spmd(nc, in_maps, core_ids=list(range(NCORES)))

    latent = np.empty((N_TOT, LAT), F32)
    for c in range(NCORES):
        latT = res.results[c]["latT"]          # [LAT, G*R_CAP]
        rows_idx, dst = scatter[c]
        latent[rows_idx] = latT.T[dst]
    return (latent, mole_frac_np)
